# revision 1
# baseline (speedup 1.0000x reference)
"""Trainium2 Bass kernel for nn_CRF: 30M-entry emission gather + exact Viterbi.

Strategy:
  Launch 1 (8 cores, SPMD): timestep-sharded emission gather. Each core
    gathers its 1024x11x14 = 157,696 f32 values from the (replicated) 30M
    table via indirect DMA (128 descriptors per instruction), then reduces
    over the 14 active features with a strided f32 left-fold reduce.
  Launch 2 (1 core): bit-exact sequential Viterbi forward pass. Carries
    m_t[j] = max_k fl(alphas_{t-1}[k] + T[k,j]) with alphas_t = fl(m_t+e_t)
    folded via one scalar_tensor_tensor op per step (exact f32 op order of
    the reference), one PE transpose, one reduce_max.
  Host: index routing, emission reassembly, backpointers/argmax, traceback
    and BIO fixup (exact f32/int elementwise post-pass).
"""

import sys

for _p in ("/opt/trn_rl_repo", "/root/.axon_site/_ro/trn_rl_repo"):
    if _p not in sys.path:
        sys.path.insert(0, _p)

import numpy as np

SEQ_LEN = 8192
L = 11
K = 14
V = 30_000_000
BOS, EOS = 9, 10
N_CORES = 8
T_CORE = SEQ_LEN // N_CORES          # 1024 timesteps per core
SLOTS_CORE = T_CORE * L              # 11264 (t, l) slots per core
P = 128
SLOTS_P = SLOTS_CORE // P            # 88 slots per partition
FCOL = SLOTS_P * K                   # 1232 gather columns per partition

_KERNELS = {}
LAST_EXEC_NS = {}
PROFILE = False


# ---------------------------------------------------------------------------
# Workarounds for this walrus build: it rejects instructions carrying more
# than one semaphore wait ("Too many sync wait commands"). Excess waits are
# hoisted onto preceding NoOps on the same engine, preserving semantics.
# ---------------------------------------------------------------------------
def _split_excess_waits(nc, max_slots=1):
    import concourse.mybir as mybir

    ctr = [0]
    for f in nc.m.functions:
        for bb in f.blocks:
            insts = bb.instructions
            new = []
            changed = False
            for inst in insts:
                si = inst.sync_info
                waits = list(si.on_wait) if si is not None else []
                ups = list(si.on_update) if si is not None else []
                if len(waits) > max_slots:
                    keep, rest = waits[:max_slots], waits[max_slots:]
                    while rest:
                        chunk, rest = rest[:max_slots], rest[max_slots:]
                        ctr[0] += 1
                        nop = mybir.InstNoOp(
                            name=f"I-wsplit-{ctr[0]}", ins=[], outs=[]
                        )
                        nop.engine = inst.engine
                        nop.sync_info = mybir.SyncInfo(on_wait=chunk, on_update=[])
                        new.append(nop)
                    inst.sync_info = mybir.SyncInfo(on_wait=keep, on_update=ups)
                    changed = True
                new.append(inst)
            if changed:
                bb.instructions = new


def _patch_tile_drain():
    import concourse.tile as tile
    import concourse.mybir as mybir
    from concourse.tile import ScopedClock

    if getattr(tile.TileContext, "_crf_drain_patched", False):
        return

    def _patched(self, tick_clock, wait_clock):
        nc = self.nc
        probe = nc.sync.nop(nofuse=True, hint="predrain_waits")
        wait_clock.add_sem_waits(
            probe.ins, ScopedClock({None: tick_clock.global_clock})
        )
        si = probe.ins.sync_info
        waits = list(si.on_wait)
        if len(waits) > 1:
            probe.ins.sync_info = mybir.SyncInfo(
                on_wait=[waits[0]], on_update=list(si.on_update)
            )
            for w in waits[1:]:
                extra = nc.sync.nop(nofuse=True, hint="predrain_waits2")
                extra.ins.sync_info = mybir.SyncInfo(on_wait=[w], on_update=[])
        nc.sync.drain()
        nc.all_engine_barrier()
        assert self.sems is not None
        popped = nc._tile_sem_poison_stack.pop()
        assert popped is self._sem_poison
        nc.clear_and_free_semaphores(list(self.sems.allocated().values()))
        nc.all_engine_barrier()

    tile.TileContext._drain_and_barrier = _patched
    tile.TileContext._crf_drain_patched = True


def _install_ntff_hook():
    """Optional: enable trace=True under axon (missing antenv.axon_hooks)."""
    import types

    try:
        import antenv.axon_hooks  # noqa: F401
        return
    except Exception:
        pass
    try:
        if "/root/.axon_site" not in sys.path:
            sys.path.insert(0, "/root/.axon_site")
        from trn_agent_boot.trn_boot import _ntff_profile_via_ctypes

        hook = [_ntff_profile_via_ctypes("/opt/axon/libaxon_pjrt.so")]
        mod = types.ModuleType("antenv.axon_hooks")
        mod.get_axon_ntff_profile_hook = lambda: hook[0]
        mod.set_axon_ntff_profile_hook = lambda h: hook.__setitem__(0, h)
        import antenv

        antenv.axon_hooks = mod
        sys.modules["antenv.axon_hooks"] = mod
    except Exception:
        pass


# ---------------------------------------------------------------------------
# Kernel builders
# ---------------------------------------------------------------------------
def _build_gather_kernel():
    import concourse.bass as bass
    import concourse.mybir as mybir
    import concourse.tile as tile

    _patch_tile_drain()
    f32d = mybir.dt.float32
    nc = bass.Bass("TRN2", num_devices=N_CORES)
    w_in = nc.declare_dram_parameter("w", [V, 1], f32d, isOutput=False)
    idx_in = nc.declare_dram_parameter(
        "idx", [P, FCOL], mybir.dt.int32, isOutput=False
    )
    em_out = nc.declare_dram_parameter("em", [P, SLOTS_P], f32d, isOutput=True)

    with tile.TileContext(nc) as tc:
        with tc.tile_pool(name="sbuf", bufs=1) as pool:
            idx_t = pool.tile([P, FCOL], mybir.dt.int32)
            gat_t = pool.tile([P, FCOL], f32d)
            emt = pool.tile([P, SLOTS_P], f32d)
            nc.gpsimd.dma_start(out=idx_t[:], in_=idx_in[:])
            for j in range(FCOL):
                nc.gpsimd.indirect_dma_start(
                    out=gat_t[:, j : j + 1],
                    out_offset=None,
                    in_=w_in[:],
                    in_offset=bass.IndirectOffsetOnAxis(
                        ap=idx_t[:, j : j + 1], axis=0
                    ),
                )
            g3 = gat_t[:].rearrange("p (s k) -> p s k", k=K)
            nc.vector.reduce_sum(out=emt[:], in_=g3, axis=mybir.AxisListType.X)
            nc.gpsimd.dma_start(out=em_out[:], in_=emt[:])

    _split_excess_waits(nc)
    return nc


def _build_dp_kernel():
    import concourse.bass as bass
    import concourse.mybir as mybir
    import concourse.tile as tile

    _patch_tile_drain()
    f32d = mybir.dt.float32
    T_STEPS = SEQ_LEN
    nc = bass.Bass("TRN2", num_devices=1)
    T_in = nc.declare_dram_parameter("T", [L, L], f32d, isOutput=False)
    emT_in = nc.declare_dram_parameter("emT", [L, T_STEPS], f32d, isOutput=False)
    tb_in = nc.declare_dram_parameter("tbos", [L, 1], f32d, isOutput=False)
    id_in = nc.declare_dram_parameter("ident", [L, L], f32d, isOutput=False)
    m_out = nc.declare_dram_parameter("mbuf", [L, T_STEPS], f32d, isOutput=True)

    with tile.TileContext(nc) as tc:
        with (
            tc.tile_pool(name="const", bufs=1) as cpool,
            tc.tile_pool(name="psum", bufs=8, space="PSUM") as ppool,
            tc.tile_pool(name="work", bufs=4) as wpool,
        ):
            T_sb = cpool.tile([L, L], f32d)
            emT_sb = cpool.tile([L, T_STEPS], f32d)
            tb_sb = cpool.tile([L, 1], f32d)
            id_sb = cpool.tile([L, L], f32d)
            mbuf = cpool.tile([L, T_STEPS], f32d)
            nc.gpsimd.dma_start(out=T_sb[:], in_=T_in[:])
            nc.gpsimd.dma_start(out=emT_sb[:], in_=emT_in[:])
            nc.gpsimd.dma_start(out=tb_sb[:], in_=tb_in[:])
            nc.gpsimd.dma_start(out=id_sb[:], in_=id_in[:])
            nc.vector.tensor_copy(out=mbuf[:, 0:1], in_=tb_sb[:])
            for t in range(1, T_STEPS):
                # sc[k,j] = fl( fl(m_{t-1}[k] + e_{t-1}[k]) + T[k,j] )
                #         = fl( alphas_{t-1}[k] + T[k,j] )   (exact ref order)
                sc = wpool.tile([L, L], f32d, tag="sc")
                nc.vector.scalar_tensor_tensor(
                    out=sc[:],
                    in0=mbuf[:, t - 1 : t].to_broadcast([L, L]),
                    scalar=emT_sb[:, t - 1 : t],
                    in1=T_sb[:],
                    op0=mybir.AluOpType.add,
                    op1=mybir.AluOpType.add,
                )
                ps = ppool.tile([L, L], f32d, tag="ps")
                nc.tensor.matmul(
                    out=ps[:], lhsT=sc[:], rhs=id_sb[:],
                    is_transpose=True, start=True, stop=True,
                )
                # m_t[j] = max_k sc[k,j]; alphas_t folded into next step's STT
                nc.vector.reduce_max(
                    out=mbuf[:, t : t + 1], in_=ps[:], axis=mybir.AxisListType.X
                )
            nc.gpsimd.dma_start(out=m_out[:], in_=mbuf[:])

    _split_excess_waits(nc)
    return nc


def _get_kernels():
    if "gather" not in _KERNELS:
        _KERNELS["gather"] = _build_gather_kernel()
    if "dp" not in _KERNELS:
        _KERNELS["dp"] = _build_dp_kernel()
    return _KERNELS["gather"], _KERNELS["dp"]


# ---------------------------------------------------------------------------
# Main entry point
# ---------------------------------------------------------------------------
def kernel(x, emission_w, transitions):
    from concourse.bass_utils import run_bass_kernel_spmd

    if PROFILE:
        _install_ntff_hook()

    x = np.asarray(x)
    w = np.ascontiguousarray(np.asarray(emission_w, dtype=np.float32))
    T = np.ascontiguousarray(np.asarray(transitions, dtype=np.float32))

    gnc, dnc = _get_kernels()

    # ---- Launch 1: emissions gather, t-sharded across 8 cores ----
    w2d = w.reshape(V, 1)
    in_maps = []
    for c in range(N_CORES):
        xs = np.ascontiguousarray(
            x[c * T_CORE : (c + 1) * T_CORE].reshape(P, FCOL).astype(np.int32)
        )
        in_maps.append({"w": w2d, "idx": xs})
    res1 = run_bass_kernel_spmd(
        gnc, in_maps, core_ids=list(range(N_CORES)), trace=PROFILE
    )
    if PROFILE:
        LAST_EXEC_NS["gather"] = res1.exec_time_ns
    em = np.concatenate(
        [res1.results[c]["em"].reshape(SLOTS_CORE) for c in range(N_CORES)]
    ).reshape(SEQ_LEN, L)

    # ---- Launch 2: exact sequential Viterbi forward (1 core) ----
    tb = np.ascontiguousarray(T[BOS][:, None])
    dp_in = {
        "T": T,
        "emT": np.ascontiguousarray(em.T),
        "tbos": tb,
        "ident": np.eye(L, dtype=np.float32),
    }
    res2 = run_bass_kernel_spmd(dnc, [dp_in], core_ids=[0], trace=PROFILE)
    if PROFILE:
        LAST_EXEC_NS["dp"] = res2.exec_time_ns
    m = res2.results[0]["mbuf"]  # (L, SEQ_LEN); m[:,0] = T[BOS]

    # alphas_t = fl(m_t + e_t)  (bit-exact: one f32 add, same as device fold)
    alphas = (m.T + em).astype(np.float32)  # (SEQ_LEN, L)

    # ---- Host post-pass: backpointers, traceback, BIO fixup (exact) ----
    sc = (alphas[:-1, :, None] + T[None, :, :]).astype(np.float32)
    sc = (sc + em[1:, None, :]).astype(np.float32)
    bps = np.argmax(sc, axis=1)  # (SEQ_LEN-1, L)

    end_scores = (alphas[-1] + T[:, EOS]).astype(np.float32)
    max_final = np.max(end_scores, keepdims=True)  # shape (1,)
    final_tag = int(np.argmax(end_scores))

    path = np.zeros(SEQ_LEN, dtype=np.int64)
    path[-1] = final_tag
    for t in range(SEQ_LEN - 2, -1, -1):
        path[t] = bps[t][path[t + 1]]

    prev = path[0]
    for t in range(1, SEQ_LEN):
        cur = path[t]
        new = cur
        if cur == 4 and prev != 3 and prev != 4:
            new = 3
        if cur == 6 and prev != 1 and prev != 6:
            new = 1
        if cur == 7 and prev != 2 and prev != 7:
            new = 2
        if cur == 8 and prev != 5 and prev != 8:
            new = 5
        path[t] = new
        prev = new

    return max_final.astype(np.float32), path.astype(np.int32)


# revision 2
# speedup vs baseline: 1.0005x; 1.0005x over previous
"""Trainium2 Bass kernel for nn_CRF: 30M-entry emission gather + exact Viterbi.

Strategy:
  Launch 1 (8 cores, SPMD): timestep-sharded emission gather. Each core
    gathers its 1024x11x14 = 157,696 f32 values from the (replicated) 30M
    table via indirect DMA (128 descriptors per instruction), then reduces
    over the 14 active features with a strided f32 left-fold reduce.
  Launch 2 (1 core): bit-exact sequential Viterbi forward pass. Carries
    m_t[j] = max_k fl(alphas_{t-1}[k] + T[k,j]) with alphas_t = fl(m_t+e_t)
    folded via one scalar_tensor_tensor op per step (exact f32 op order of
    the reference), one PE transpose, one reduce_max.
  Host: index routing, emission reassembly, backpointers/argmax, traceback
    and BIO fixup (exact f32/int elementwise post-pass).
"""

import sys

for _p in ("/opt/trn_rl_repo", "/root/.axon_site/_ro/trn_rl_repo"):
    if _p not in sys.path:
        sys.path.insert(0, _p)

import numpy as np

SEQ_LEN = 8192
L = 11
K = 14
V = 30_000_000
BOS, EOS = 9, 10
N_CORES = 8
T_CORE = SEQ_LEN // N_CORES          # 1024 timesteps per core
SLOTS_CORE = T_CORE * L              # 11264 (t, l) slots per core
P = 128
SLOTS_P = SLOTS_CORE // P            # 88 slots per partition
FCOL = SLOTS_P * K                   # 1232 gather columns per partition

_KERNELS = {}
LAST_EXEC_NS = {}
PROFILE = False


# ---------------------------------------------------------------------------
# Workarounds for this walrus build: it rejects instructions carrying more
# than one semaphore wait ("Too many sync wait commands"). Excess waits are
# hoisted onto preceding NoOps on the same engine, preserving semantics.
# ---------------------------------------------------------------------------
def _split_excess_waits(nc, max_slots=1):
    import concourse.mybir as mybir

    ctr = [0]
    for f in nc.m.functions:
        for bb in f.blocks:
            insts = bb.instructions
            new = []
            changed = False
            for inst in insts:
                si = inst.sync_info
                waits = list(si.on_wait) if si is not None else []
                ups = list(si.on_update) if si is not None else []
                if len(waits) > max_slots:
                    keep, rest = waits[:max_slots], waits[max_slots:]
                    while rest:
                        chunk, rest = rest[:max_slots], rest[max_slots:]
                        ctr[0] += 1
                        nop = mybir.InstNoOp(
                            name=f"I-wsplit-{ctr[0]}", ins=[], outs=[]
                        )
                        nop.engine = inst.engine
                        nop.sync_info = mybir.SyncInfo(on_wait=chunk, on_update=[])
                        new.append(nop)
                    inst.sync_info = mybir.SyncInfo(on_wait=keep, on_update=ups)
                    changed = True
                new.append(inst)
            if changed:
                bb.instructions = new


def _patch_tile_drain():
    import concourse.tile as tile
    import concourse.mybir as mybir
    from concourse.tile import ScopedClock

    if getattr(tile.TileContext, "_crf_drain_patched", False):
        return

    def _patched(self, tick_clock, wait_clock):
        nc = self.nc
        probe = nc.sync.nop(nofuse=True, hint="predrain_waits")
        wait_clock.add_sem_waits(
            probe.ins, ScopedClock({None: tick_clock.global_clock})
        )
        si = probe.ins.sync_info
        waits = list(si.on_wait)
        if len(waits) > 1:
            probe.ins.sync_info = mybir.SyncInfo(
                on_wait=[waits[0]], on_update=list(si.on_update)
            )
            for w in waits[1:]:
                extra = nc.sync.nop(nofuse=True, hint="predrain_waits2")
                extra.ins.sync_info = mybir.SyncInfo(on_wait=[w], on_update=[])
        nc.sync.drain()
        nc.all_engine_barrier()
        assert self.sems is not None
        popped = nc._tile_sem_poison_stack.pop()
        assert popped is self._sem_poison
        nc.clear_and_free_semaphores(list(self.sems.allocated().values()))
        nc.all_engine_barrier()

    tile.TileContext._drain_and_barrier = _patched
    tile.TileContext._crf_drain_patched = True


def _install_ntff_hook():
    """Optional: enable trace=True under axon (missing antenv.axon_hooks)."""
    import types

    try:
        import antenv.axon_hooks  # noqa: F401
        return
    except Exception:
        pass
    try:
        if "/root/.axon_site" not in sys.path:
            sys.path.insert(0, "/root/.axon_site")
        from trn_agent_boot.trn_boot import _ntff_profile_via_ctypes

        hook = [_ntff_profile_via_ctypes("/opt/axon/libaxon_pjrt.so")]
        mod = types.ModuleType("antenv.axon_hooks")
        mod.get_axon_ntff_profile_hook = lambda: hook[0]
        mod.set_axon_ntff_profile_hook = lambda h: hook.__setitem__(0, h)
        import antenv

        antenv.axon_hooks = mod
        sys.modules["antenv.axon_hooks"] = mod
    except Exception:
        pass


# ---------------------------------------------------------------------------
# Kernel builders
# ---------------------------------------------------------------------------
def _build_gather_kernel():
    import concourse.bass as bass
    import concourse.mybir as mybir
    import concourse.tile as tile

    _patch_tile_drain()
    f32d = mybir.dt.float32
    nc = bass.Bass("TRN2", num_devices=N_CORES)
    w_in = nc.declare_dram_parameter("w", [V, 1], f32d, isOutput=False)
    idx_in = nc.declare_dram_parameter(
        "idx", [P, FCOL], mybir.dt.int32, isOutput=False
    )
    em_out = nc.declare_dram_parameter("em", [P, SLOTS_P], f32d, isOutput=True)

    with tile.TileContext(nc) as tc:
        with tc.tile_pool(name="sbuf", bufs=1) as pool:
            idx_t = pool.tile([P, FCOL], mybir.dt.int32)
            gat_t = pool.tile([P, FCOL], f32d)
            emt = pool.tile([P, SLOTS_P], f32d)
            nc.gpsimd.dma_start(out=idx_t[:], in_=idx_in[:])
            for j in range(FCOL):
                nc.gpsimd.indirect_dma_start(
                    out=gat_t[:, j : j + 1],
                    out_offset=None,
                    in_=w_in[:],
                    in_offset=bass.IndirectOffsetOnAxis(
                        ap=idx_t[:, j : j + 1], axis=0
                    ),
                )
            g3 = gat_t[:].rearrange("p (s k) -> p s k", k=K)
            nc.vector.reduce_sum(out=emt[:], in_=g3, axis=mybir.AxisListType.X)
            nc.gpsimd.dma_start(out=em_out[:], in_=emt[:])

    _split_excess_waits(nc)
    return nc


def _build_dp_kernel():
    import concourse.bass as bass
    import concourse.mybir as mybir
    import concourse.tile as tile

    _patch_tile_drain()
    f32d = mybir.dt.float32
    T_STEPS = SEQ_LEN
    nc = bass.Bass("TRN2", num_devices=1)
    T_in = nc.declare_dram_parameter("T", [L, L], f32d, isOutput=False)
    emT_in = nc.declare_dram_parameter("emT", [L, T_STEPS], f32d, isOutput=False)
    tb_in = nc.declare_dram_parameter("tbos", [L, 1], f32d, isOutput=False)
    id_in = nc.declare_dram_parameter("ident", [L, L], f32d, isOutput=False)
    m_out = nc.declare_dram_parameter("mbuf", [L, T_STEPS], f32d, isOutput=True)

    with tile.TileContext(nc) as tc:
        with (
            tc.tile_pool(name="const", bufs=1) as cpool,
            tc.tile_pool(name="psum", bufs=8, space="PSUM") as ppool,
            tc.tile_pool(name="work", bufs=4) as wpool,
        ):
            T_sb = cpool.tile([L, L], f32d)
            emT_sb = cpool.tile([L, T_STEPS], f32d)
            tb_sb = cpool.tile([L, 1], f32d)
            id_sb = cpool.tile([L, L], f32d)
            mbuf = cpool.tile([L, T_STEPS], f32d)
            nc.gpsimd.dma_start(out=T_sb[:], in_=T_in[:])
            nc.gpsimd.dma_start(out=emT_sb[:], in_=emT_in[:])
            nc.gpsimd.dma_start(out=tb_sb[:], in_=tb_in[:])
            nc.gpsimd.dma_start(out=id_sb[:], in_=id_in[:])
            nc.vector.tensor_copy(out=mbuf[:, 0:1], in_=tb_sb[:])
            for t in range(1, T_STEPS):
                # sc[k,j] = fl( fl(m_{t-1}[k] + e_{t-1}[k]) + T[k,j] )
                #         = fl( alphas_{t-1}[k] + T[k,j] )   (exact ref order)
                sc = wpool.tile([L, L], f32d, tag="sc")
                nc.vector.scalar_tensor_tensor(
                    out=sc[:],
                    in0=mbuf[:, t - 1 : t].to_broadcast([L, L]),
                    scalar=emT_sb[:, t - 1 : t],
                    in1=T_sb[:],
                    op0=mybir.AluOpType.add,
                    op1=mybir.AluOpType.add,
                )
                ps = ppool.tile([L, L], f32d, tag="ps")
                nc.tensor.matmul(
                    out=ps[:], lhsT=sc[:], rhs=id_sb[:],
                    is_transpose=True, start=True, stop=True,
                )
                # m_t[j] = max_k sc[k,j]; alphas_t folded into next step's STT
                nc.vector.reduce_max(
                    out=mbuf[:, t : t + 1], in_=ps[:], axis=mybir.AxisListType.X
                )
            nc.gpsimd.dma_start(out=m_out[:], in_=mbuf[:])

    _split_excess_waits(nc)
    return nc


def _get_kernels():
    if "gather" not in _KERNELS:
        _KERNELS["gather"] = _build_gather_kernel()
    if "dp" not in _KERNELS:
        _KERNELS["dp"] = _build_dp_kernel()
    return _KERNELS["gather"], _KERNELS["dp"]


# ---------------------------------------------------------------------------
# Main entry point
# ---------------------------------------------------------------------------
def kernel(x, emission_w, transitions):
    from concourse.bass_utils import run_bass_kernel_spmd

    if PROFILE:
        _install_ntff_hook()

    x = np.asarray(x)
    w = np.ascontiguousarray(np.asarray(emission_w, dtype=np.float32))
    T = np.ascontiguousarray(np.asarray(transitions, dtype=np.float32))

    gnc, dnc = _get_kernels()

    # ---- Launch 1: emissions gather, t-sharded across 8 cores ----
    w2d = w.reshape(V, 1)
    in_maps = []
    for c in range(N_CORES):
        xs = np.ascontiguousarray(
            x[c * T_CORE : (c + 1) * T_CORE].reshape(P, FCOL).astype(np.int32)
        )
        in_maps.append({"w": w2d, "idx": xs})
    res1 = run_bass_kernel_spmd(
        gnc, in_maps, core_ids=list(range(N_CORES)), trace=PROFILE
    )
    if PROFILE:
        LAST_EXEC_NS["gather"] = res1.exec_time_ns
    em = np.concatenate(
        [res1.results[c]["em"].reshape(SLOTS_CORE) for c in range(N_CORES)]
    ).reshape(SEQ_LEN, L)

    # ---- Launch 2: exact sequential Viterbi forward (1 core) ----
    tb = np.ascontiguousarray(T[BOS][:, None])
    dp_in = {
        "T": T,
        "emT": np.ascontiguousarray(em.T),
        "tbos": tb,
        "ident": np.eye(L, dtype=np.float32),
    }
    res2 = run_bass_kernel_spmd(dnc, [dp_in], core_ids=[0], trace=PROFILE)
    if PROFILE:
        LAST_EXEC_NS["dp"] = res2.exec_time_ns
    m = res2.results[0]["mbuf"]  # (L, SEQ_LEN); m[:,0] = T[BOS]

    # alphas_t = fl(m_t + e_t)  (bit-exact: one f32 add, same as device fold)
    alphas = (m.T + em).astype(np.float32)  # (SEQ_LEN, L)

    # ---- Host post-pass: backpointers, traceback, BIO fixup (exact) ----
    sc = (alphas[:-1, :, None] + T[None, :, :]).astype(np.float32)
    sc = (sc + em[1:, None, :]).astype(np.float32)
    bps = np.argmax(sc, axis=1)  # (SEQ_LEN-1, L)

    end_scores = (alphas[-1] + T[:, EOS]).astype(np.float32)
    max_final = np.max(end_scores, keepdims=True)  # shape (1,)
    final_tag = int(np.argmax(end_scores))

    path = np.zeros(SEQ_LEN, dtype=np.int64)
    path[-1] = final_tag
    for t in range(SEQ_LEN - 2, -1, -1):
        path[t] = bps[t][path[t + 1]]

    prev = path[0]
    for t in range(1, SEQ_LEN):
        cur = path[t]
        new = cur
        if cur == 4 and prev != 3 and prev != 4:
            new = 3
        if cur == 6 and prev != 1 and prev != 6:
            new = 1
        if cur == 7 and prev != 2 and prev != 7:
            new = 2
        if cur == 8 and prev != 5 and prev != 8:
            new = 5
        path[t] = new
        prev = new

    # Match the reference's path dtype: jnp.argmax yields int64 when x64 is
    # enabled (x arrives int64), int32 otherwise.
    path_dtype = np.int64 if x.dtype == np.int64 else np.int32
    return max_final.astype(np.float32), path.astype(path_dtype)


# revision 4
# speedup vs baseline: 1.2895x; 1.2889x over previous
"""Trainium2 Bass kernel for nn_CRF: 30M-entry emission gather + exact Viterbi.

Strategy:
  Launch 1 (8 cores, SPMD): timestep-sharded emission gather. Each core
    gathers its 1024x11x14 = 157,696 f32 values from the (replicated) 30M
    table via indirect DMA (128 descriptors per instruction), then reduces
    over the 14 active features with a strided f32 left-fold reduce.
  Launch 2 (1 core): bit-exact sequential Viterbi forward pass. Carries
    m_t[j] = max_k fl(alphas_{t-1}[k] + T[k,j]) with alphas_t = fl(m_t+e_t)
    folded via one scalar_tensor_tensor op per step (exact f32 op order of
    the reference), one PE transpose, one reduce_max.
  Host: index routing, emission reassembly, backpointers/argmax, traceback
    and BIO fixup (exact f32/int elementwise post-pass).
"""

import sys

for _p in ("/opt/trn_rl_repo", "/root/.axon_site/_ro/trn_rl_repo"):
    if _p not in sys.path:
        sys.path.insert(0, _p)

import numpy as np

SEQ_LEN = 8192
L = 11
K = 14
V = 30_000_000
BOS, EOS = 9, 10
N_CORES = 8
T_CORE = SEQ_LEN // N_CORES          # 1024 timesteps per core
SLOTS_CORE = T_CORE * L              # 11264 (t, l) slots per core
P = 128
SLOTS_P = SLOTS_CORE // P            # 88 slots per partition
FCOL = SLOTS_P * K                   # 1232 gather columns per partition

_KERNELS = {}
LAST_EXEC_NS = {}
PROFILE = False


# ---------------------------------------------------------------------------
# Workarounds for this walrus build: it rejects instructions carrying more
# than one semaphore wait ("Too many sync wait commands"). Excess waits are
# hoisted onto preceding NoOps on the same engine, preserving semantics.
# ---------------------------------------------------------------------------
def _split_excess_waits(nc, max_slots=1):
    import concourse.mybir as mybir

    ctr = [0]
    for f in nc.m.functions:
        for bb in f.blocks:
            insts = bb.instructions
            new = []
            changed = False
            for inst in insts:
                si = inst.sync_info
                waits = list(si.on_wait) if si is not None else []
                ups = list(si.on_update) if si is not None else []
                if len(waits) > max_slots:
                    keep, rest = waits[:max_slots], waits[max_slots:]
                    while rest:
                        chunk, rest = rest[:max_slots], rest[max_slots:]
                        ctr[0] += 1
                        nop = mybir.InstNoOp(
                            name=f"I-wsplit-{ctr[0]}", ins=[], outs=[]
                        )
                        nop.engine = inst.engine
                        nop.sync_info = mybir.SyncInfo(on_wait=chunk, on_update=[])
                        new.append(nop)
                    inst.sync_info = mybir.SyncInfo(on_wait=keep, on_update=ups)
                    changed = True
                new.append(inst)
            if changed:
                bb.instructions = new


def _patch_tile_drain():
    import concourse.tile as tile
    import concourse.mybir as mybir
    from concourse.tile import ScopedClock

    if getattr(tile.TileContext, "_crf_drain_patched", False):
        return

    def _patched(self, tick_clock, wait_clock):
        nc = self.nc
        probe = nc.sync.nop(nofuse=True, hint="predrain_waits")
        wait_clock.add_sem_waits(
            probe.ins, ScopedClock({None: tick_clock.global_clock})
        )
        si = probe.ins.sync_info
        waits = list(si.on_wait)
        if len(waits) > 1:
            probe.ins.sync_info = mybir.SyncInfo(
                on_wait=[waits[0]], on_update=list(si.on_update)
            )
            for w in waits[1:]:
                extra = nc.sync.nop(nofuse=True, hint="predrain_waits2")
                extra.ins.sync_info = mybir.SyncInfo(on_wait=[w], on_update=[])
        nc.sync.drain()
        nc.all_engine_barrier()
        assert self.sems is not None
        popped = nc._tile_sem_poison_stack.pop()
        assert popped is self._sem_poison
        nc.clear_and_free_semaphores(list(self.sems.allocated().values()))
        nc.all_engine_barrier()

    tile.TileContext._drain_and_barrier = _patched
    tile.TileContext._crf_drain_patched = True


def _install_ntff_hook():
    """Optional: enable trace=True under axon (missing antenv.axon_hooks)."""
    import types

    try:
        import antenv.axon_hooks  # noqa: F401
        return
    except Exception:
        pass
    try:
        if "/root/.axon_site" not in sys.path:
            sys.path.insert(0, "/root/.axon_site")
        from trn_agent_boot.trn_boot import _ntff_profile_via_ctypes

        hook = [_ntff_profile_via_ctypes("/opt/axon/libaxon_pjrt.so")]
        mod = types.ModuleType("antenv.axon_hooks")
        mod.get_axon_ntff_profile_hook = lambda: hook[0]
        mod.set_axon_ntff_profile_hook = lambda h: hook.__setitem__(0, h)
        import antenv

        antenv.axon_hooks = mod
        sys.modules["antenv.axon_hooks"] = mod
    except Exception:
        pass


# ---------------------------------------------------------------------------
# Kernel builders
# ---------------------------------------------------------------------------
def _build_gather_kernel():
    import concourse.bass as bass
    import concourse.mybir as mybir
    import concourse.tile as tile

    _patch_tile_drain()
    f32d = mybir.dt.float32
    nc = bass.Bass("TRN2", num_devices=N_CORES)
    w_in = nc.declare_dram_parameter("w", [V, 1], f32d, isOutput=False)
    idx_in = nc.declare_dram_parameter(
        "idx", [P, FCOL], mybir.dt.int32, isOutput=False
    )
    em_out = nc.declare_dram_parameter("em", [P, SLOTS_P], f32d, isOutput=True)

    with tile.TileContext(nc) as tc:
        with tc.tile_pool(name="sbuf", bufs=1) as pool:
            idx_t = pool.tile([P, FCOL], mybir.dt.int32)
            gat_t = pool.tile([P, FCOL], f32d)
            emt = pool.tile([P, SLOTS_P], f32d)
            nc.gpsimd.dma_start(out=idx_t[:], in_=idx_in[:])
            for j in range(FCOL):
                nc.gpsimd.indirect_dma_start(
                    out=gat_t[:, j : j + 1],
                    out_offset=None,
                    in_=w_in[:],
                    in_offset=bass.IndirectOffsetOnAxis(
                        ap=idx_t[:, j : j + 1], axis=0
                    ),
                )
            g3 = gat_t[:].rearrange("p (s k) -> p s k", k=K)
            nc.vector.reduce_sum(out=emt[:], in_=g3, axis=mybir.AxisListType.X)
            nc.gpsimd.dma_start(out=em_out[:], in_=emt[:])

    _split_excess_waits(nc)
    return nc


def _build_dp_kernel():
    import concourse.bass as bass
    import concourse.mybir as mybir
    import concourse.tile as tile

    _patch_tile_drain()
    f32d = mybir.dt.float32
    T_STEPS = SEQ_LEN
    NEG = -3.0e38
    nc = bass.Bass("TRN2", num_devices=1)
    T_in = nc.declare_dram_parameter("T", [L, L], f32d, isOutput=False)
    emT_in = nc.declare_dram_parameter("emT", [L, T_STEPS], f32d, isOutput=False)
    tb_in = nc.declare_dram_parameter("tbos", [L, 1], f32d, isOutput=False)
    m_out = nc.declare_dram_parameter("mbuf", [L, T_STEPS], f32d, isOutput=True)

    with tile.TileContext(nc) as tc:
        with tc.tile_pool(name="const", bufs=1) as cpool:
            T_sb = cpool.tile([L, L], f32d)
            emT_sb = cpool.tile([L, T_STEPS], f32d)
            tb_sb = cpool.tile([L, 1], f32d)
            mbuf = cpool.tile([32, T_STEPS], f32d)
            sc32 = cpool.tile([32, 32], f32d)
            nc.gpsimd.dma_start(out=T_sb[:], in_=T_in[:])
            nc.gpsimd.dma_start(out=emT_sb[:], in_=emT_in[:])
            nc.gpsimd.dma_start(out=tb_sb[:], in_=tb_in[:])
            nc.vector.memset(sc32[:], NEG)
            nc.vector.tensor_copy(out=mbuf[0:L, 0:1], in_=tb_sb[:])
            for t in range(1, T_STEPS):
                # sc[k,j] = fl( fl(m_{t-1}[k] + e_{t-1}[k]) + T[k,j] )
                #         = fl( alphas_{t-1}[k] + T[k,j] )   (exact ref order)
                nc.vector.scalar_tensor_tensor(
                    out=sc32[0:L, 0:L],
                    in0=mbuf[0:L, t - 1 : t].to_broadcast([L, L]),
                    scalar=emT_sb[:, t - 1 : t],
                    in1=T_sb[:],
                    op0=mybir.AluOpType.add,
                    op1=mybir.AluOpType.add,
                )
                # m_t[j] = max_k sc[k,j] — transpose fused into the reduce
                # (one DVE op; rows/cols 11..31 of sc32 stay at NEG).
                nc.vector.tensor_reduce(
                    out=mbuf[:, t : t + 1], in_=sc32[:],
                    op=mybir.AluOpType.max, axis=mybir.AxisListType.X,
                    apply_transpose=True,
                )
            nc.gpsimd.dma_start(out=m_out[:], in_=mbuf[0:L, :])

    _split_excess_waits(nc)
    return nc


def _get_kernels():
    if "gather" not in _KERNELS:
        _KERNELS["gather"] = _build_gather_kernel()
    if "dp" not in _KERNELS:
        _KERNELS["dp"] = _build_dp_kernel()
    return _KERNELS["gather"], _KERNELS["dp"]


# ---------------------------------------------------------------------------
# Main entry point
# ---------------------------------------------------------------------------
def kernel(x, emission_w, transitions):
    from concourse.bass_utils import run_bass_kernel_spmd

    if PROFILE:
        _install_ntff_hook()

    x = np.asarray(x)
    w = np.ascontiguousarray(np.asarray(emission_w, dtype=np.float32))
    T = np.ascontiguousarray(np.asarray(transitions, dtype=np.float32))

    gnc, dnc = _get_kernels()

    # ---- Launch 1: emissions gather, t-sharded across 8 cores ----
    w2d = w.reshape(V, 1)
    in_maps = []
    for c in range(N_CORES):
        xs = np.ascontiguousarray(
            x[c * T_CORE : (c + 1) * T_CORE].reshape(P, FCOL).astype(np.int32)
        )
        in_maps.append({"w": w2d, "idx": xs})
    res1 = run_bass_kernel_spmd(
        gnc, in_maps, core_ids=list(range(N_CORES)), trace=PROFILE
    )
    if PROFILE:
        LAST_EXEC_NS["gather"] = res1.exec_time_ns
    em = np.concatenate(
        [res1.results[c]["em"].reshape(SLOTS_CORE) for c in range(N_CORES)]
    ).reshape(SEQ_LEN, L)

    # ---- Launch 2: exact sequential Viterbi forward (1 core) ----
    tb = np.ascontiguousarray(T[BOS][:, None])
    dp_in = {
        "T": T,
        "emT": np.ascontiguousarray(em.T),
        "tbos": tb,
    }
    res2 = run_bass_kernel_spmd(dnc, [dp_in], core_ids=[0], trace=PROFILE)
    if PROFILE:
        LAST_EXEC_NS["dp"] = res2.exec_time_ns
    m = res2.results[0]["mbuf"]  # (L, SEQ_LEN); m[:,0] = T[BOS]

    # alphas_t = fl(m_t + e_t)  (bit-exact: one f32 add, same as device fold)
    alphas = (m.T + em).astype(np.float32)  # (SEQ_LEN, L)

    # ---- Host post-pass: backpointers, traceback, BIO fixup (exact) ----
    sc = (alphas[:-1, :, None] + T[None, :, :]).astype(np.float32)
    sc = (sc + em[1:, None, :]).astype(np.float32)
    bps = np.argmax(sc, axis=1)  # (SEQ_LEN-1, L)

    end_scores = (alphas[-1] + T[:, EOS]).astype(np.float32)
    max_final = np.max(end_scores, keepdims=True)  # shape (1,)
    final_tag = int(np.argmax(end_scores))

    path = np.zeros(SEQ_LEN, dtype=np.int64)
    path[-1] = final_tag
    for t in range(SEQ_LEN - 2, -1, -1):
        path[t] = bps[t][path[t + 1]]

    prev = path[0]
    for t in range(1, SEQ_LEN):
        cur = path[t]
        new = cur
        if cur == 4 and prev != 3 and prev != 4:
            new = 3
        if cur == 6 and prev != 1 and prev != 6:
            new = 1
        if cur == 7 and prev != 2 and prev != 7:
            new = 2
        if cur == 8 and prev != 5 and prev != 8:
            new = 5
        path[t] = new
        prev = new

    # Match the reference's path dtype: jnp.argmax yields int64 when x64 is
    # enabled (x arrives int64), int32 otherwise.
    path_dtype = np.int64 if x.dtype == np.int64 else np.int32
    return max_final.astype(np.float32), path.astype(path_dtype)
